# revision 29
# baseline (speedup 1.0000x reference)
"""Attn_LSTM Trainium2 kernel — 8-core data-parallel Bass/Tile implementation.

Model (per reference): 1-layer LSTM encoder over L=96 steps, then T=24
attention-decoder steps. B=4096 sharded 512/core across 8 NeuronCores;
weights replicated.

Device-side design (driven by measured engine rates):
  * PE matmuls all-bf16 (fp32 PSUM accumulation), gates paired (i,f)/(g,o)
    into [128,512] matmuls with K-stacked inputs ([h;x] K=72 encoder,
    [emb;ctx] K=128 + h K=64 decoder).
  * DVE: only TensorTensor (1 elem/cyc/lane fp32, 2/cyc pure-bf16) and
    TensorScalar (2/cyc) ops — scalar_tensor_tensor and tensor_tensor_scan
    are microcoded ~8-20x slower on this DVE and are avoided. GpSimd (Pool)
    is erratic/slow and unused for compute.
  * Cell: sigmoid/tanh activations on the ACT engine (cost ~0.84ns/col,
    independent of partition count, so partition-paired gates are free);
    cell math is 4 pure-bf16 tensor_tensor ops. States bf16.
  * Attention context: softmax numerators e=exp(z) from one ACT call; then
    ctx = (sum_l e_l*enc_l)/(sum_l e_l) via ONE fused bf16 multiply over
    [128, NCH, L, H+1] (e broadcast along h; broadcasts are free) and a
    7-op bf16 binary ADD TREE over l. A ones-column at h=H yields the
    softmax denominator from the same pass.
  * The local walrus build accepts at most ONE semaphore wait per
    instruction; legalize_waits() splits extra waits onto same-engine NoOps.
"""

import numpy as np
import ml_dtypes

import concourse.bass as bass
import concourse.tile as tile
from concourse import mybir
from concourse.masks import make_identity
from concourse.bass_utils import run_bass_kernel_spmd

H = 64
C = 8
L = 96
T = 24
B = 4096
NCORES = 8
BS = B // NCORES          # 512 batch per core
NCH = BS // 128           # 4 partition chunks per core

F32 = mybir.dt.float32
BF16 = mybir.dt.bfloat16
NPBF = ml_dtypes.bfloat16
AF = mybir.ActivationFunctionType
ALU = mybir.AluOpType


def _legalize_waits(nc):
    """This walrus build rejects >1 sem wait per instruction; split extras
    onto same-engine NoOps placed immediately before."""
    cnt = 0
    for bb in nc.main_func.blocks:
        new = []
        for inst in bb.instructions:
            si = inst.sync_info
            if si is not None and len(si.on_wait) > 1:
                waits = list(si.on_wait)
                for w in waits[:-1]:
                    nop = mybir.InstNoOp(name=f"wsplit-{cnt}", ins=[], outs=[])
                    cnt += 1
                    nop.engine = inst.engine
                    nop.sync_info = mybir.SyncInfo(on_wait=[w], on_update=[])
                    new.append(nop)
                inst.sync_info = mybir.SyncInfo(
                    on_wait=[waits[-1]], on_update=list(si.on_update))
            new.append(inst)
        bb.instructions = new
    return cnt


def _tts_raw(nc, eng, out, data0, data1, initial, op0, op1):
    """tensor_tensor_scan without the 2D-shape assert (kept for probes)."""
    return eng.add_instruction(
        mybir.InstTensorScalarPtr(
            name=nc.get_next_instruction_name(),
            is_tensor_tensor_scan=True,
            is_scalar_tensor_tensor=True,
            op0=op0,
            op1=op1,
            ins=[
                eng.lower_ap(data0),
                eng.lower_ap_or_imm(initial),
                eng.lower_ap(data1),
            ],
            outs=[eng.lower_ap(out)],
        )
    )


def _build_program():
    nc = bass.Bass("TRN2", target_bir_lowering=False, debug=False,
                   num_devices=NCORES)

    def din(name, shape, dt=BF16):
        return nc.dram_tensor(name, list(shape), dt, kind="ExternalInput").ap()

    xT = din("xT", (L, C, BS))                  # normalized, transposed, bf16
    enc_w = din("enc_w", (72, 2, 128))          # rows 0:64=Whh_p.T, 64:72=Wih_p.T
    dec_ec = din("dec_ec", (128, 2, 128))       # rows 0:64=wie_p.T, 64:128=wic_p.T
    dec_hh = din("dec_hh", (64, 2, 128))        # dec_Whh_p.T
    w_we = din("w_we", (H, L))                  # attn emb-part We.T
    w_wh = din("w_wh", (H + 1, L))              # attn h-part Wh.T + bias row
    w_emb = din("w_emb", (H, H))                # (emb_W@out_W).T
    w_out = din("w_out", (H, C))                # out_W.T
    b_enc = din("b_enc", (128, 2), F32)         # act biases per pair
    b_dec = din("b_dec", (128, 2), F32)
    emb_bh = din("emb_bh", (H, 1), F32)         # emb_W@out_b + emb_b
    emb0 = din("emb0", (H, 1), F32)             # relu(emb_b)  (t=0 embedding)
    out_b = din("out_b", (C, 1), F32)

    preds = nc.dram_tensor("preds", [T, C, BS], F32, kind="ExternalOutput").ap()

    with tile.TileContext(nc) as tc:
        with (
            tc.tile_pool(name="state", bufs=1) as st,
            tc.tile_pool(name="outp", bufs=2) as outp,
            tc.tile_pool(name="ctxp", bufs=1) as ctxp,
            tc.tile_pool(name="gps", bufs=1, space="PSUM") as gps,
            tc.tile_pool(name="mps", bufs=1, space="PSUM") as mps,
        ):
            # ---------- persistent tiles ----------
            ident_f = st.tile([128, 128], F32)
            make_identity(nc, ident_f[:])
            ident = st.tile([128, 128], BF16)
            nc.scalar.copy(ident[:], ident_f[:])

            w_enc_sb = st.tile([72, 2, 128], BF16)
            w_ec_sb = st.tile([128, 2, 128], BF16)
            w_hh_sb = st.tile([64, 2, 128], BF16)
            w_we_sb = st.tile([H, L], BF16)
            w_wh_sb = st.tile([H + 1, L], BF16)
            w_emb_sb = st.tile([H, H], BF16)
            w_out_sb = st.tile([H, C], BF16)
            b_enc_sb = st.tile([128, 2], F32)
            b_dec_sb = st.tile([128, 2], F32)
            emb_bh_sb = st.tile([H, 1], F32)
            emb0_sb = st.tile([H, 1], F32)
            out_b_sb = st.tile([C, 1], F32)
            for tl, ap in ((w_enc_sb, enc_w), (w_ec_sb, dec_ec),
                           (w_hh_sb, dec_hh), (w_we_sb, w_we),
                           (w_wh_sb, w_wh), (w_emb_sb, w_emb),
                           (w_out_sb, w_out), (b_enc_sb, b_enc),
                           (b_dec_sb, b_dec), (emb_bh_sb, emb_bh),
                           (emb0_sb, emb0), (out_b_sb, out_b)):
                nc.gpsimd.dma_start(tl[:], ap[:])

            # recurrent state: h (bf16) with ones row 64 (attn bias);
            # c lives at partitions 64:128 so the two-input DVE ops have
            # partition-aligned operands (f/o sit at rows 64:128 of the
            # pair tiles); outputs may shift partitions freely.
            h_T = st.tile([H + 1, BS], BF16)
            cb = st.tile([128, BS], BF16)      # c at rows 64:128
            nc.vector.memset(h_T[:], 0.0)
            nc.vector.memset(cb[64:128, :], 0.0)
            nc.vector.memset(h_T[H : H + 1, :], 1.0)

            # encoder outputs [b, chunk, h(65), l] bf16; row H = ones.
            # l innermost: the e-broadcast in the ctx multiply must sit on a
            # NON-inner dim (inner stride-0 drops DVE bf16 from 2 to 1
            # elem/cycle).
            enc_plus = st.tile([128, NCH, H + 1, L], BF16)
            nc.vector.memset(enc_plus[:, :, H, :], 1.0)

            # encoder combined rhs: rows 0:64 = h, 64:72 = x (ping-pong)
            xh = [st.tile([72, BS], BF16, name=f"xh{p}") for p in range(2)]
            for p in range(2):
                nc.vector.memset(xh[p][0:H, :], 0.0)

            # act outputs + cell temps
            if_sb = st.tile([128, BS], BF16)
            go_sb = st.tile([128, BS], BF16)
            t1_sb = st.tile([H, BS], BF16)
            t2_sb = st.tile([H, BS], BF16)
            tc_sb = st.tile([128, BS], BF16)   # used rows 64:128

            # decoder tiles
            dmy_sb = st.tile([128, 1], BF16)       # table-prefetch target
            ec_sb = st.tile([128, BS], BF16)       # rows 0:64 emb, 64:128 ctx
            e_sb = st.tile([128, NCH, L], BF16)
            rec_sb = st.tile([128, NCH], F32)
            ctx_ch = st.tile([128, NCH, H], BF16)

            # PSUM: encoder uses full-width pair tiles; the decoder halves
            # get exclusive [128, 256] tiles so each (pair, half) owns its
            # accumulation group (two concurrently-open groups must not share
            # a bank).
            gate_ps = [gps.tile([128, BS], F32, tag=f"gp{p}", name=f"gp{p}")
                       for p in range(2)]


            def lstm_cell(bias_tile):
                """pair PSUMs -> activations -> c/h update (h into dst)."""
                nc.scalar.activation(go_sb[0:H, :], gate_ps[1][0:H, :],
                                     AF.Tanh, bias=bias_tile[0:H, 1:2])
                nc.scalar.activation(if_sb[:], gate_ps[0][:], AF.Sigmoid,
                                     bias=bias_tile[:, 0:1])
                nc.scalar.activation(go_sb[H:128, :], gate_ps[1][H:128, :],
                                     AF.Sigmoid, bias=bias_tile[H:128, 1:2])
                nc.vector.tensor_mul(t1_sb[:], if_sb[0:H, :], go_sb[0:H, :])
                nc.vector.tensor_mul(t2_sb[:], if_sb[H:128, :], cb[H:128, :])
                nc.vector.tensor_add(cb[H:128, :], t1_sb[:], t2_sb[:])
                nc.scalar.activation(tc_sb[H:128, :], cb[H:128, :], AF.Tanh)

            def h_out(dst):
                nc.vector.tensor_mul(dst, go_sb[H:128, :], tc_sb[H:128, :])

            # ------------------ encoder ------------------
            nc.sync.dma_start(xh[0][H:72, :], xT[0])
            for l in range(L):
                if l + 1 < L:
                    nc.sync.dma_start(xh[(l + 1) % 2][H:72, :], xT[l + 1])
                for p in (1, 0):
                    nc.tensor.matmul(gate_ps[p][:], w_enc_sb[:, p, :],
                                     xh[l % 2][:], start=True, stop=True)
                lstm_cell(b_enc_sb)
                holder = h_T if l == L - 1 else xh[(l + 1) % 2]
                h_out(holder[0:H, :])
                # store h (transposed back to [b, h]) into enc_plus[:,:,0:H,l].
                # Transpose via a REAL matmul against identity so the PSUM
                # result is fp32 (4-byte aligned at any l offset); 4 steps
                # land strided in one PSUM tile and one copy per 4 steps
                # amortizes the strided enc write.
                if l % 4 == 0:
                    tp4 = mps.tile([128, NCH, H, 4], F32, tag="big")
                for ci in range(NCH):
                    nc.tensor.matmul(tp4[:, ci, :, l % 4],
                                     holder[0:H, 128 * ci : 128 * (ci + 1)],
                                     ident[0:H, 0:H], start=True, stop=True)
                if l % 4 == 3:
                    nc.scalar.copy(enc_plus[:, :, 0:H, l - 3 : l + 1], tp4[:])

            # ------------------ decoder ------------------
            for t in range(T):
                # embedding into ec rows 0:64 (bf16)
                if t == 0:
                    nc.vector.tensor_scalar(
                        out=ec_sb[0:H, :],
                        in0=emb0_sb[:, 0:1].broadcast_to((H, BS)),
                        scalar1=0.0, scalar2=None, op0=ALU.add)
                else:
                    emb_ps = mps.tile([H, BS], F32, tag="emb")
                    nc.tensor.matmul(emb_ps[:], w_emb_sb[:], h_T[0:H, :],
                                     start=True, stop=True)
                    nc.scalar.activation(ec_sb[0:H, :], emb_ps[:], AF.Relu,
                                         bias=emb_bh_sb[:, 0:1])

                # attention scores -> e = exp(z)
                zd_ps = mps.tile([128, NCH, L], F32, tag="zd")
                for ci in range(NCH):
                    sl = slice(128 * ci, 128 * (ci + 1))
                    nc.tensor.matmul(zd_ps[:, ci, :], ec_sb[0:H, sl],
                                     w_we_sb[:], start=True, stop=False)
                    nc.tensor.matmul(zd_ps[:, ci, :], h_T[:, sl],
                                     w_wh_sb[:], start=False, stop=True)
                nc.scalar.activation(e_sb[:], zd_ps[:], AF.Exp)
                # prefetch the sigmoid/tanh table while the DVE runs the ctx
                # multiply+tree: the exp->sigmoid table swap then never sits
                # in the gates' serial window.
                nc.scalar.activation(dmy_sb[:], b_dec_sb[:, 0:1], AF.Sigmoid)
                # gate h-parts: h is ready now, accumulate early (off-chain)
                for p in (1, 0):
                    nc.tensor.matmul(gate_ps[p][:], w_hh_sb[:, p, :],
                                     h_T[0:H, :], start=True, stop=False)

                # ctx: one fused bf16 multiply + bf16 add-tree over l
                P = ctxp.tile([128, NCH, H + 1, L], BF16, tag="P")
                qa = ctxp.tile([128, NCH, H + 1, 48], BF16, tag="qa")
                qb = ctxp.tile([128, NCH, H + 1, 24], BF16, tag="qb")
                qc = ctxp.tile([128, NCH, H + 1, 12], BF16, tag="qc")
                qd = ctxp.tile([128, NCH, H + 1, 6], BF16, tag="qd")
                qe = ctxp.tile([128, NCH, H + 1, 3], BF16, tag="qe")
                Rt = ctxp.tile([128, NCH, H + 1], BF16, tag="Rt")
                St = ctxp.tile([128, NCH, H + 1], BF16, tag="St")
                e_bc = e_sb[:].unsqueeze(2).broadcast_to((128, NCH, H + 1, L))
                nc.vector.tensor_mul(P[:], enc_plus[:], e_bc)
                nc.vector.tensor_add(qa[:], P[:, :, :, 0:48], P[:, :, :, 48:96])
                nc.vector.tensor_add(qb[:], qa[:, :, :, 0:24], qa[:, :, :, 24:48])
                nc.vector.tensor_add(qc[:], qb[:, :, :, 0:12], qb[:, :, :, 12:24])
                nc.vector.tensor_add(qd[:], qc[:, :, :, 0:6], qc[:, :, :, 6:12])
                nc.vector.tensor_add(qe[:], qd[:, :, :, 0:3], qd[:, :, :, 3:6])
                nc.vector.tensor_add(Rt[:], qe[:, :, :, 0], qe[:, :, :, 1])
                nc.vector.tensor_add(St[:], Rt[:], qe[:, :, :, 2])

                ctxT_ps = mps.tile([H, BS], BF16, tag="big")
                for ci in range(NCH):
                    nc.vector.reciprocal(rec_sb[:, ci : ci + 1],
                                         St[:, ci, H : H + 1])
                    nc.vector.tensor_scalar(
                        out=ctx_ch[:, ci, :], in0=St[:, ci, 0:H],
                        scalar1=rec_sb[:, ci : ci + 1], scalar2=None,
                        op0=ALU.mult)
                    nc.tensor.transpose(ctxT_ps[:, 128 * ci : 128 * (ci + 1)],
                                        ctx_ch[:, ci, :], ident[:])
                nc.scalar.copy(ec_sb[H:128, :], ctxT_ps[:])

                # decoder LSTM cell: ec-part accumulates onto the hoisted
                # h-part
                for p in (1, 0):
                    nc.tensor.matmul(gate_ps[p][:], w_ec_sb[:, p, :],
                                     ec_sb[:], start=False, stop=True)
                lstm_cell(b_dec_sb)
                h_out(h_T[0:H, :])
                # prefetch the exp table during the cell window for the next
                # step's score exponentials
                nc.scalar.activation(dmy_sb[:], b_dec_sb[:, 0:1], AF.Exp)

                # prediction -> output store
                pred_ps = mps.tile([H, BS], F32, tag="emb")
                nc.tensor.matmul(pred_ps[0:C, :], w_out_sb[:], h_T[0:H, :],
                                 start=True, stop=True)
                po = outp.tile([C, BS], F32, tag="po")
                nc.scalar.activation(po[:], pred_ps[0:C, :], AF.Identity,
                                     bias=out_b_sb[:, 0:1])
                nc.sync.dma_start(preds[t], po[:])

    _legalize_waits(nc)
    return nc


_NC_CACHE = []


def _get_nc():
    if not _NC_CACHE:
        _NC_CACHE.append(_build_program())
    return _NC_CACHE[0]


def _bf(x):
    return np.ascontiguousarray(np.asarray(x, np.float32).astype(NPBF))


def _prep_weights(i):
    """Host-side packing. Gate pairs: p0=(i,f), p1=(g,o) in pytorch row order."""
    Wih = np.asarray(i["enc_Wih"], np.float32)
    Whh = np.asarray(i["enc_Whh"], np.float32)
    be = np.asarray(i["enc_bih"] + i["enc_bhh"], np.float32)

    enc_w = np.zeros((72, 2, 128), np.float32)
    for p in range(2):
        r = slice(128 * p, 128 * (p + 1))
        enc_w[0:64, p, :] = Whh[r].T
        enc_w[64:72, p, :] = Wih[r].T

    emb_W = np.asarray(i["emb_W"], np.float32)
    emb_b = np.asarray(i["emb_b"], np.float32)
    attn_W = np.asarray(i["attn_W"], np.float32)
    attn_b = np.asarray(i["attn_b"], np.float32)
    comb_W = np.asarray(i["comb_W"], np.float32)
    comb_b = np.asarray(i["comb_b"], np.float32)
    dWih = np.asarray(i["dec_Wih"], np.float32)
    dWhh = np.asarray(i["dec_Whh"], np.float32)
    bd = (np.asarray(i["dec_bih"] + i["dec_bhh"], np.float32)
          + dWih @ comb_b)
    out_W = np.asarray(i["out_W"], np.float32)
    out_bv = np.asarray(i["out_b"], np.float32)

    wie = dWih @ comb_W[:, :H]
    wic = dWih @ comb_W[:, H:]
    dec_ec = np.zeros((128, 2, 128), np.float32)
    dec_hh = np.zeros((64, 2, 128), np.float32)
    for p in range(2):
        r = slice(128 * p, 128 * (p + 1))
        dec_ec[0:64, p, :] = wie[r].T
        dec_ec[64:128, p, :] = wic[r].T
        dec_hh[:, p, :] = dWhh[r].T

    w_wh = np.zeros((H + 1, L), np.float32)
    w_wh[0:H] = attn_W[:, H:].T
    w_wh[H] = attn_b

    def bias_pack(b):
        out = np.zeros((128, 2), np.float32)
        out[:, 0] = b[0:128]
        out[0:64, 1] = b[128:192]
        out[64:128, 1] = b[192:256]
        return out

    return dict(
        enc_w=_bf(enc_w), dec_ec=_bf(dec_ec), dec_hh=_bf(dec_hh),
        w_we=_bf(attn_W[:, :H].T), w_wh=_bf(w_wh),
        w_emb=_bf((emb_W @ out_W).T), w_out=_bf(out_W.T),
        b_enc=bias_pack(be), b_dec=bias_pack(bd),
        emb_bh=(emb_W @ out_bv + emb_b).reshape(H, 1).astype(np.float32),
        emb0=np.maximum(emb_b, 0.0).reshape(H, 1).astype(np.float32),
        out_b=out_bv.reshape(C, 1).astype(np.float32),
    )


def kernel(**inputs):
    x_enc = np.asarray(inputs["x_enc"], np.float32)
    seq_last = x_enc[:, -1:, :]                       # [B, 1, C]
    x = x_enc - seq_last                              # [B, L, C]

    weights = _prep_weights({k: np.asarray(v) for k, v in inputs.items()
                             if k not in ("x_enc", "x_mark_enc", "x_dec",
                                          "x_mark_dec")})

    core_ids = list(range(NCORES))
    in_maps = []
    for ci in core_ids:
        xs = x[ci * BS : (ci + 1) * BS]               # [BS, L, C]
        xTc = np.ascontiguousarray(
            xs.transpose(1, 2, 0).astype(NPBF))       # [L, C, BS] bf16
        m = dict(weights)
        m["xT"] = xTc
        in_maps.append(m)

    nc = _get_nc()
    res = run_bass_kernel_spmd(nc, in_maps, core_ids)
    global LAST_RESULTS
    LAST_RESULTS = res

    out = np.empty((B, T, C), np.float32)
    for ci in core_ids:
        p = res.results[ci]["preds"]                  # [T, C, BS]
        out[ci * BS : (ci + 1) * BS] = p.transpose(2, 0, 1)
    out += seq_last
    return out
